# revision 32
# baseline (speedup 1.0000x reference)
"""Trainium2 Bass kernel for nn_CroAttention (cosine cross-attention block).

Computation (per (b,t) pair, 64 pairs total):
  qh  = l2norm_heads(q @ Wq.T + bq)          (256, 8, 64)
  k,v = l2norm_heads(kv @ Wkv.T + bkv)       (512, 8, 64) each
  att = softmax(qh @ kh.T / 8)  per head     (cosine scores in [-1/8, 1/8])
  x   = att @ vh  -> merge heads
  out = x @ Wm.T + bm + q

Sharding: data-parallel over the 64 fused (b,t) pairs -> 8 pairs per core.

Device dataflow (v2 — bf16 operands, ones-column softmax sums):
  - host feeds qT [c, lq], kvT [c, lk] per pair (bf16) + bf16 weights
  - Q/K projections emit qh^T [(h d), lq], kh^T [(h d), lk]; bias is fused
    into the PSUM->SBUF copy on ACT; squares run on the idle GPSIMD; the
    per-head squared norms accumulate via a block-indicator matmul
  - V projection emits vh [lk, h, 65]: 64 value dims + a ones column, so
    the PV matmul's row 64 delivers the softmax denominators for free
  - scores^T[k, q] per head -> exp via ACT (cosine scores bounded, no max
    subtraction); PV output [65, lq] is evacuated to SBUF pair-tiles
    (2 heads per 128-partition tile) by ACT; row 64 gathered into [8, lq]
  - one batched reciprocal (bits-log seed + 1 Newton) + 4 pair-packed
    broadcast matmuls normalize x; out proj packs 2 heads per K-chunk
    (K=128) -> out^T = WmT.T @ x^T + bm + qT, un-transposed on the host.

All matmul operands are bf16 (1 cycle/row on TRN2 at any N); PSUM
accumulation stays fp32. bf16 rounding (~0.4% rel) is far inside the 2e-2
tolerance, and the fp32 residual stream dilutes it further.
"""

import sys

sys.path.insert(0, "/opt/trn_rl_repo")

import numpy as np
import ml_dtypes

import concourse.bass as bass
import concourse.mybir as mybir
import concourse.tile as tile
from concourse import bacc
from concourse.bass_utils import run_bass_kernel_spmd

F32 = mybir.dt.float32
BF16 = mybir.dt.bfloat16
I32 = mybir.dt.int32
AF = mybir.ActivationFunctionType
NPBF16 = ml_dtypes.bfloat16

LN2 = 0.6931471805599453
MU = 0.0450


def _bits_exp_coefs(p):
    """exp(a*bits(s) + b) ~= s**p via the float-bits logarithm."""
    return p * LN2 / (2 ** 23), -p * LN2 * (127 - MU)

C = 512          # channels
H = 8            # heads
D = 64           # head dim
LQ = 256         # query length
LK = 512         # kv length
P = 8            # (b,t) pairs per core
NCORES = 8
NCH = 4          # c split into 4 chunks of 128
EPS2 = 1e-24     # eps^2 for max(norm^2, .) ; sqrt(1e-24) = 1e-12 = torch eps


def _patch_act_tables():
    """Pin the ACT table set to natural_log_exp_and_others (Identity/Exp)."""
    orig = bacc.get_activation_tables

    def patched(arch):
        tabs = orig(arch)
        name = "natural_log_exp_and_others"
        if name in tabs:
            return {name: tabs[name]}
        return tabs

    bacc.get_activation_tables = patched


def build_program():
    _patch_act_tables()
    nc = bacc.Bacc(
        "TRN2", target_bir_lowering=False, debug=False, enable_asserts=False
    )

    # ---- DRAM I/O (per core) ----
    qT_d = nc.dram_tensor("qT", [P * C, LQ], BF16, kind="ExternalInput").ap()
    kvT_d = nc.dram_tensor("kvT", [P * C, LK], BF16, kind="ExternalInput").ap()
    wqT_d = nc.dram_tensor("wqT", [C, C], BF16, kind="ExternalInput").ap()
    wkT_d = nc.dram_tensor("wkT", [C, C], BF16, kind="ExternalInput").ap()
    wvT_d = nc.dram_tensor("wvT", [C, C], BF16, kind="ExternalInput").ap()
    # wm2[d2, hp, o] = Wm[o, hp*128 + d2]: two heads stacked per K-chunk
    wm2_d = nc.dram_tensor("wm2", [128, NCH, C], BF16, kind="ExternalInput").ap()
    bq_d = nc.dram_tensor("bq", [128, NCH], F32, kind="ExternalInput").ap()
    bk_d = nc.dram_tensor("bk", [128, NCH], F32, kind="ExternalInput").ap()
    bm_d = nc.dram_tensor("bm", [128, NCH], F32, kind="ExternalInput").ap()
    bv_d = nc.dram_tensor("bv", [1, C], F32, kind="ExternalInput").ap()
    # ind[h, o] = 1.0 if o // 64 == h else 0  (bcast inv-norm rows -> o lanes)
    ind_d = nc.dram_tensor("ind", [H, C], BF16, kind="ExternalInput").ap()
    # blk[o, h] = ind.T (accumulate per-head sums of squares)
    blk_d = nc.dram_tensor("blk", [C, H], BF16, kind="ExternalInput").ap()
    # ind2[k, p, m] = 1 if (m<64 and k==2p) or (m>=64 and k==2p+1)
    ind2_d = nc.dram_tensor("ind2", [H, NCH, 128], BF16, kind="ExternalInput").ap()
    # sel[k, h, m] = 1 if k == 64 and m == h (route sums row 64 -> row h)
    sel_d = nc.dram_tensor("sel", [D + 1, H, H], BF16, kind="ExternalInput").ap()
    outT_d = nc.dram_tensor("outT", [P * C, LQ], F32, kind="ExternalOutput").ap()

    with tile.TileContext(nc) as tc:
        with (
            tc.tile_pool(name="singles", bufs=1) as singles,
            tc.tile_pool(name="qin", bufs=3) as qin_pool,
            tc.tile_pool(name="kvin", bufs=3) as kvin_pool,
            tc.tile_pool(name="qh", bufs=2) as qh_pool,
            tc.tile_pool(name="kh", bufs=2) as kh_pool,
            tc.tile_pool(name="vh", bufs=2) as vh_pool,
            tc.tile_pool(name="sq", bufs=5) as sq_pool,
            tc.tile_pool(name="inv", bufs=2) as inv_pool,
            tc.tile_pool(name="nv", bufs=2) as nv_pool,
            tc.tile_pool(name="es", bufs=9) as es_pool,
            tc.tile_pool(name="rc", bufs=2) as rc_pool,
            tc.tile_pool(name="xsb", bufs=2) as xsb_pool,
            tc.tile_pool(name="outs", bufs=3) as out_pool,
            tc.tile_pool(name="ps", bufs=8, space="PSUM") as ps_pool,
        ):
            # ---- persistent tiles ----
            w_sb = {}
            for name, d in (("wq", wqT_d), ("wk", wkT_d), ("wv", wvT_d)):
                t = singles.tile([128, NCH, C], BF16, tag=f"w_{name}")
                nc.sync.dma_start(out=t, in_=d.rearrange("(kc p) o -> p kc o", p=128))
                w_sb[name] = t
            wm2_sb = singles.tile([128, NCH, C], BF16, tag="w_wm")
            nc.sync.dma_start(out=wm2_sb, in_=wm2_d)
            bq_sb = singles.tile([128, NCH], F32, tag="bq")
            nc.sync.dma_start(out=bq_sb, in_=bq_d)
            bk_sb = singles.tile([128, NCH], F32, tag="bk")
            nc.sync.dma_start(out=bk_sb, in_=bk_d)
            bm_sb = singles.tile([128, NCH], F32, tag="bm")
            nc.sync.dma_start(out=bm_sb, in_=bm_d)
            bv_sb = singles.tile([128, C], F32, tag="bv")
            nc.sync.dma_start(out=bv_sb, in_=bv_d.to_broadcast([128, C]))
            ind_sb = singles.tile([H, C], BF16, tag="ind")
            nc.sync.dma_start(out=ind_sb, in_=ind_d)
            blk_sb = singles.tile([128, NCH, H], BF16, tag="blk")
            nc.sync.dma_start(out=blk_sb, in_=blk_d.rearrange("(j p) h -> p j h", p=128))
            ind2_sb = singles.tile([H, NCH, 128], BF16, tag="ind2")
            nc.sync.dma_start(out=ind2_sb, in_=ind2_d)
            sel_sb = singles.tile([D + 1, H, H], BF16, tag="sel")
            nc.sync.dma_start(out=sel_sb, in_=sel_d)
            # bias rows for the bits-log exp seeds (s^-1/2 and s^-1)
            a_h, b_h = _bits_exp_coefs(-0.5)
            a_r, b_r = _bits_exp_coefs(-1.0)
            bh_sb = singles.tile([128, 1], F32, tag="bh")
            nc.vector.memset(bh_sb, b_h)
            br_sb = singles.tile([128, 1], F32, tag="br")
            nc.vector.memset(br_sb, b_r)
            eps2_bits = int(np.float32(EPS2).view(np.int32))

            def rsqrt_rounds(out_r, s_ps, pool, npart, n, tag, p, nr, clamp):
                """out_r = s_ps ** p via bits-log exp seed + nr Newton steps.
                s_ps is an f32 AP (PSUM or SBUF); clamp applies max(s, eps^2)
                in the int domain (valid for positive floats)."""
                a, b, bias = (a_h, b_h, bh_sb) if p == -0.5 else (a_r, b_r, br_sb)
                if p == -1.0:
                    # pure-DVE seed: y_bits = MAGIC - bits (fast inverse);
                    # avoids an ACT hop on the softmax critical path
                    yb = pool.tile([npart, n], I32, tag=tag + "yb")
                    nc.vector.tensor_scalar(
                        out=yb, in0=s_ps.bitcast(I32),
                        scalar1=-1, scalar2=0x7EF311C3,
                        op0=mybir.AluOpType.mult, op1=mybir.AluOpType.add,
                    )
                    y = yb.bitcast(F32)
                else:
                    bits_f = pool.tile([npart, n], F32, tag=tag + "b")
                    if clamp:
                        nc.vector.tensor_scalar(
                            out=bits_f, in0=s_ps.bitcast(I32),
                            scalar1=eps2_bits, scalar2=None,
                            op0=mybir.AluOpType.max,
                        )
                    else:
                        nc.vector.tensor_copy(bits_f, s_ps.bitcast(I32))
                    y = pool.tile([npart, n], F32, tag=tag + "y")
                    nc.scalar.activation(
                        out=y, in_=bits_f, func=AF.Exp, scale=a, bias=bias[0:npart, :]
                    )
                for it in range(nr):
                    t = pool.tile([npart, n], F32, tag=tag + "t")
                    if p == -0.5:
                        # t = s*y^2 ; y <- y*(1.5 - 0.5 t)
                        nc.vector.tensor_mul(t, y, y)
                        nc.vector.tensor_mul(t, t, s_ps)
                        nc.vector.tensor_scalar(
                            out=t, in0=t, scalar1=-0.5, scalar2=1.5,
                            op0=mybir.AluOpType.mult, op1=mybir.AluOpType.add,
                        )
                    else:
                        # t = s*y ; y <- y*(2 - t)
                        nc.vector.tensor_mul(t, y, s_ps)
                        nc.vector.tensor_scalar(
                            out=t, in0=t, scalar1=-1.0, scalar2=2.0,
                            op0=mybir.AluOpType.mult, op1=mybir.AluOpType.add,
                        )
                    last = it == nr - 1
                    yn = out_r if last else pool.tile([npart, n], F32, tag=tag + "y")
                    nc.vector.tensor_mul(yn, t, y)
                    y = yn

            # ---- pipelined pair stages -------------------------------
            # PE program order per iteration interleaves pair i's
            # dependency-free projection matmuls between pair (i-1)'s
            # attention phases, so the in-order PE queue never parks at
            # the head of a norm/softmax latency chain.

            def stage_load(i):
                st = {}
                st["q"] = qin_pool.tile([128, NCH, LQ], BF16, tag="qin", name="q_sb")
                nc.sync.dma_start(
                    out=st["q"],
                    in_=qT_d[i * C:(i + 1) * C, :].rearrange("(j p) l -> p j l", p=128),
                )
                st["kv"] = kvin_pool.tile([128, NCH, LK], BF16, tag="kvin", name="kv_sb")
                nc.sync.dma_start(
                    out=st["kv"],
                    in_=kvT_d[i * C:(i + 1) * C, :].rearrange("(j p) l -> p j l", p=128),
                )
                st["i"] = i
                return st

            def proj_mm(st, wname, in_sb, n, hname):
                """Projection matmuls + fused bias (ACT) + square (Pool) +
                per-head norm accumulation (PE). No normalize yet."""
                h_sb = (qh_pool if hname == "q" else kh_pool).tile(
                    [128, NCH, n], BF16, tag=hname + "h", name="h_sb"
                )
                norm_ps = ps_pool.tile([H, n], F32, tag="ps", name="norm_ps")
                w = w_sb[wname]
                for j in range(NCH):
                    ps = ps_pool.tile([128, n], F32, tag="ps", name="ps")
                    for kc in range(NCH):
                        nc.tensor.matmul(
                            ps,
                            lhsT=w[:, kc, j * 128:(j + 1) * 128],
                            rhs=in_sb[:, kc, :],
                            start=(kc == 0),
                            stop=(kc == NCH - 1),
                        )
                    nc.scalar.activation(
                        out=h_sb[:, j, :], in_=ps, func=AF.Identity,
                        bias=(bq_sb if hname == "q" else bk_sb)[:, j:j + 1],
                    )
                    sq = sq_pool.tile([128, n], BF16, tag="sq" + hname, name="sq")
                    nc.gpsimd.tensor_mul(sq, h_sb[:, j, :], h_sb[:, j, :])
                    nc.tensor.matmul(
                        norm_ps,
                        lhsT=blk_sb[:, j, :],
                        rhs=sq,
                        start=(j == 0),
                        stop=(j == NCH - 1),
                    )
                st[hname + "h"] = h_sb
                st["nps" + hname] = norm_ps

            def stage_proj(st):
                proj_mm(st, "wq", st["q"], LQ, "q")
                proj_mm(st, "wk", st["kv"], LK, "k")
                # V projection (natural layout) + ones column + norms
                vh_sb = vh_pool.tile([128, NCH, H, D + 1], BF16, tag="vh", name="vh_sb")
                nc.vector.memset(vh_sb[:, :, :, D:D + 1], 1.0)
                nv_all = nv_pool.tile([128, NCH, H], F32, tag="nv", name="nv_all")
                for j in range(NCH):  # lk chunk
                    ps = ps_pool.tile([128, C], F32, tag="ps", name="ps")
                    for kc in range(NCH):
                        nc.tensor.matmul(
                            ps,
                            lhsT=st["kv"][:, kc, j * 128:(j + 1) * 128],
                            rhs=w_sb["wv"][:, kc, :],
                            start=(kc == 0),
                            stop=(kc == NCH - 1),
                        )
                    nc.vector.tensor_add(
                        vh_sb[:, j, :, 0:D],
                        ps.rearrange("p (h d) -> p h d", h=H),
                        bv_sb.rearrange("p (h d) -> p h d", h=H),
                    )
                st["vh"] = vh_sb
                st["nv"] = nv_all

            def stage_proj_dve(st):
                """Slack-tolerant DVE chains (V norms, q/k inv-norm seeds) —
                emitted after the previous pair's finish so its rec chain
                is not queued behind them on DVE."""
                for hname, n in (("q", LQ), ("k", LK)):
                    inv_sb = inv_pool.tile(
                        [H, n], BF16, tag="inv" + hname, name="inv_sb"
                    )
                    rsqrt_rounds(
                        inv_sb, st["nps" + hname], inv_pool, H, n,
                        "inv" + hname, p=-0.5, nr=1, clamp=True,
                    )
                    st["inv" + hname] = inv_sb
                vh_sb = st["vh"]
                nv_all = st["nv"]
                for j in range(NCH):
                    sqv = sq_pool.tile([128, H, D], BF16, tag="sqv", name="sqv")
                    nc.gpsimd.tensor_mul(
                        sqv, vh_sb[:, j, :, 0:D], vh_sb[:, j, :, 0:D]
                    )
                    nc.vector.reduce_sum(
                        nv_all[:, j, :], sqv, axis=mybir.AxisListType.X
                    )
                nvr = nv_pool.tile([128, NCH, H], F32, tag="nvr", name="nvr")
                rsqrt_rounds(
                    nvr.rearrange("p a b -> p (a b)"),
                    nv_all.rearrange("p a b -> p (a b)"),
                    nv_pool, 128, NCH * H, "nv",
                    p=-0.5, nr=1, clamp=True,
                )
                for j in range(NCH):
                    nc.gpsimd.tensor_mul(
                        vh_sb[:, j, :, 0:D],
                        vh_sb[:, j, :, 0:D],
                        nvr[:, j, :].broadcast_to([128, H, D]),
                    )

            def stage_norm(st):
                """Broadcast inv norms over head partitions and scale q/k.
                Interleaved q0,k0,q1,k1,... so the first heads' chunks are
                ready before the next pair's scores matmuls need them."""
                for j in range(NCH):
                    for hname, n in (("q", LQ), ("k", LK)):
                        h_sb = st[hname + "h"]
                        inv_sb = st["inv" + hname]
                        bc = ps_pool.tile([128, n], F32, tag="ps", name="bc")
                        nc.tensor.matmul(
                            bc,
                            lhsT=ind_sb[:, j * 128:(j + 1) * 128],
                            rhs=inv_sb,
                            start=True, stop=True,
                        )
                        nc.vector.tensor_mul(h_sb[:, j, :], h_sb[:, j, :], bc)

            def stage_scores(st):
                """All heads' scores matmuls + exp."""
                qh_sb, kh_sb = st["qh"], st["kh"]
                es_all = []
                for h in range(H):
                    jh, ph = h // 2, (h % 2) * D
                    es_sb = es_pool.tile([128, NCH, LQ], BF16, tag="es", name="es_sb")
                    for jkk in range(NCH // 2):  # pairs of lk chunks
                        ps_s = ps_pool.tile([128, 2, LQ], F32, tag="ps", name="ps_s")
                        for s in range(2):
                            jk = 2 * jkk + s
                            nc.tensor.matmul(
                                ps_s[:, s, :],
                                lhsT=kh_sb[ph:ph + D, jh, jk * 128:(jk + 1) * 128],
                                rhs=qh_sb[ph:ph + D, jh, :],
                                start=True, stop=True,
                            )
                        # att = exp(scores / sqrt(D)); cosine scores bounded
                        nc.scalar.activation(
                            out=es_sb[:, 2 * jkk:2 * jkk + 2, :], in_=ps_s,
                            func=AF.Exp, scale=0.125,
                        )
                    es_all.append(es_sb)
                st["es"] = es_all

            def stage_pv(st):
                """All heads' PV + evac; route sums rows to one PSUM tile."""
                vh_sb = st["vh"]
                es_all = st["es"]
                x65 = []
                for h in range(H):
                    es_sb = es_all[h]
                    ps_x = ps_pool.tile([D + 1, LQ], F32, tag="ps", name="ps_x")
                    for jk in range(NCH):
                        nc.tensor.matmul(
                            ps_x,
                            lhsT=vh_sb[:, jk, h, :],
                            rhs=es_sb[:, jk, :],
                            start=(jk == 0),
                            stop=(jk == NCH - 1),
                        )
                    # evacuate x~ (+ its sums row 64) to SBUF
                    xs = xsb_pool.tile([D + 1, LQ], BF16, tag=f"x{h}", name="xs")
                    nc.scalar.activation(out=xs, in_=ps_x, func=AF.Identity)
                    x65.append(xs)
                # route the 8 sums rows into one [8, lq] PSUM tile
                sums_ps = ps_pool.tile([H, LQ], F32, tag="ps", name="sums_ps")
                for h in range(H):
                    nc.tensor.matmul(
                        sums_ps,
                        lhsT=sel_sb[:, h, :],
                        rhs=x65[h],
                        start=(h == 0),
                        stop=(h == H - 1),
                    )
                st["sums"] = sums_ps
                st["x65"] = x65

            def stage_finish(st):
                """Softmax denominators, x normalize, out proj, store."""
                rec_r = rc_pool.tile([H, LQ], BF16, tag="rcr", name="rec_r")
                rsqrt_rounds(
                    rec_r, st["sums"], rc_pool, H, LQ, "rec",
                    p=-1.0, nr=1, clamp=False,
                )
                x65 = st["x65"]
                xsb = [
                    xsb_pool.tile([128, LQ], BF16, tag=f"xsb{hp}", name=f"xsb{hp}")
                    for hp in range(NCH)
                ]
                for hp in range(NCH):
                    bc = ps_pool.tile([128, LQ], F32, tag="ps", name="bc")
                    nc.tensor.matmul(
                        bc, lhsT=ind2_sb[:, hp, :], rhs=rec_r,
                        start=True, stop=True,
                    )
                    for s in range(2):
                        h = 2 * hp + s
                        nc.vector.tensor_mul(
                            xsb[hp][s * D:(s + 1) * D, :],
                            x65[h][0:D, :],
                            bc[s * D:(s + 1) * D, :],
                        )
                out_sb = out_pool.tile([128, NCH, LQ], F32, tag="outs", name="out_sb")
                for jo in range(NCH):
                    ps_o = ps_pool.tile([128, LQ], F32, tag="ps", name="ps_o")
                    for hp in range(NCH):  # K chunks of 128 (two heads each)
                        nc.tensor.matmul(
                            ps_o,
                            lhsT=wm2_sb[:, hp, jo * 128:(jo + 1) * 128],
                            rhs=xsb[hp],
                            start=(hp == 0),
                            stop=(hp == NCH - 1),
                        )
                    # out = ps_o + bm + qT  (fused bias + residual)
                    nc.vector.scalar_tensor_tensor(
                        out=out_sb[:, jo, :],
                        in0=ps_o,
                        scalar=bm_sb[:, jo:jo + 1],
                        in1=st["q"][:, jo, :],
                        op0=mybir.AluOpType.add,
                        op1=mybir.AluOpType.add,
                    )
                i = st["i"]
                nc.sync.dma_start(
                    out=outT_d[i * C:(i + 1) * C, :].rearrange(
                        "(j p) l -> p j l", p=128
                    ),
                    in_=out_sb,
                )

            prev = None
            for i in range(P):
                cur = stage_load(i)
                if prev is not None:
                    stage_scores(prev)
                    stage_pv(prev)
                stage_proj(cur)
                if prev is not None:
                    stage_finish(prev)
                stage_proj_dve(cur)
                stage_norm(cur)
                prev = cur
            stage_scores(prev)
            stage_pv(prev)
            stage_finish(prev)

    nc.compile()
    return nc


_NC_CACHE = None


def _get_program():
    global _NC_CACHE
    if _NC_CACHE is None:
        _NC_CACHE = build_program()
    return _NC_CACHE


def prep_in_maps(q, kv, Wq, bq, Wkv, bkv, Wm, bm):
    q = np.ascontiguousarray(np.asarray(q, dtype=np.float32))
    kv = np.ascontiguousarray(np.asarray(kv, dtype=np.float32))
    b, t, lq, c = q.shape
    lk = kv.shape[2]
    npairs = b * t
    per_core = npairs // NCORES

    # host-side transposes / weight prep (not on the device critical path)
    qT = np.ascontiguousarray(
        q.reshape(npairs, lq, c).transpose(0, 2, 1).astype(NPBF16)
    )  # [64, c, lq]
    kvT = np.ascontiguousarray(
        kv.reshape(npairs, lk, c).transpose(0, 2, 1).astype(NPBF16)
    )  # [64, c, lk]
    wqT = np.ascontiguousarray(np.asarray(Wq, np.float32).T.astype(NPBF16))
    wkT = np.ascontiguousarray(np.asarray(Wkv[:C], np.float32).T.astype(NPBF16))
    wvT = np.ascontiguousarray(np.asarray(Wkv[C:], np.float32).T.astype(NPBF16))
    # wm2[d2, hp, o] = Wm[o, hp*128 + d2]
    wm2 = np.ascontiguousarray(
        np.asarray(Wm, np.float32).T.reshape(NCH, 128, C).transpose(1, 0, 2)
        .astype(NPBF16)
    )
    bq_t = np.ascontiguousarray(np.asarray(bq, np.float32).reshape(NCH, 128).T)
    bk_t = np.ascontiguousarray(
        np.asarray(bkv[:C], np.float32).reshape(NCH, 128).T
    )
    bv_t = np.ascontiguousarray(np.asarray(bkv[C:], np.float32).reshape(1, C))
    bm_t = np.ascontiguousarray(np.asarray(bm, np.float32).reshape(NCH, 128).T)
    ind = np.zeros((H, C), NPBF16)
    for h in range(H):
        ind[h, h * D:(h + 1) * D] = 1.0
    blk = np.ascontiguousarray(ind.T)
    ind2 = np.zeros((H, NCH, 128), NPBF16)
    for hp in range(NCH):
        ind2[2 * hp, hp, 0:D] = 1.0
        ind2[2 * hp + 1, hp, D:128] = 1.0
    sel = np.zeros((D + 1, H, H), NPBF16)
    for h in range(H):
        sel[D, h, h] = 1.0

    in_maps = []
    for core in range(NCORES):
        s = core * per_core
        e = s + per_core
        in_maps.append({
            "qT": qT[s:e].reshape(per_core * C, lq),
            "kvT": kvT[s:e].reshape(per_core * C, lk),
            "wqT": wqT, "wkT": wkT, "wvT": wvT, "wm2": wm2,
            "bq": bq_t, "bk": bk_t, "bv": bv_t, "bm": bm_t,
            "ind": ind, "blk": blk, "ind2": ind2, "sel": sel,
        })
    return in_maps, (b, t, lq, c, per_core)


def kernel(q, kv, Wq, bq, Wkv, bkv, Wm, bm):
    in_maps, (b, t, lq, c, per_core) = prep_in_maps(q, kv, Wq, bq, Wkv, bkv, Wm, bm)
    nc = _get_program()
    res = run_bass_kernel_spmd(nc, in_maps, core_ids=list(range(NCORES)))
    outT = np.concatenate(
        [res.results[core]["outT"].reshape(per_core, C, lq) for core in range(NCORES)],
        axis=0,
    )  # [64, c, lq]
    out = outT.transpose(0, 2, 1).reshape(b, t, lq, c)
    return np.ascontiguousarray(out)
